# revision 3
# baseline (speedup 1.0000x reference)
"""CLIP causal attention (B=8, T=1024, E=768, H=12) on 8 TRN2 NeuronCores.

Strategy: pure data-parallel over batch — core b handles x[b] end to end,
no collectives. All compute in transposed space (embed on partitions):

  X' = x_b^T                       [768, 1024]  (host pre-transposed, bf16)
  Q' = Wq^T @ X' (+bq)             [768, 1024]  lhsT = Wq as stored
  K' = Wk^T @ X' (+bk)             [768, 1024]
  V  = X'^T @ Wv (+bv)             [1024, 768]  lhsT = X' blocks (j on partitions)
  per head h (KQ orientation, j on partitions, i free):
     S'[j,i] = K'_h[:,jblk]^T @ Q'_h          (K=64)
       QS2/KS2 are partition-SWAPPED copies of Q'/K' (halves exchanged via
       SBUF->SBUF DMA): each pack's two S matmuls read alternating copies ->
       adjacent issues target disjoint PE row groups and run CONCURRENTLY.
     P' = exp(S' * 1/8)  (no max-subtraction: |S'/8| <= ~7, exact-safe)
     causal: skip fully-masked blocks, restrict to valid cols, tri-mask diag
     PV matmuls run one pack late (software pipeline): each pack's exp()
       hides under the next pack's S matmuls.
     O_aug[d,i] = sum_j Vaug_h[j,d]^T @ P'    (Vaug ones column -> row 64 =
                                               softmax denominator)
     O'_h = O_aug[0:64] * broadcast(1/denom)
  out = (O'^T @ Wo) + bo           [1024, 768]

v2 restructure (startup was 52us of DMA-starved projections in v1):
  - The single HW DGE queue round-robins packets across ALL pending DMAs,
    so nothing lands until the whole input stream is done.  Input DMAs are
    now split into an URGENT set (Wq/Wk nt0 blocks, X', V-weights for
    heads 0-1) issued immediately, and a BULK set gated behind compute
    progress via 1-element DVE corner-copies (WAW dep on the DMA dst makes
    the sync engine hold the issue until the gate tile is produced).
  - Wq/Wk are host-relaid nt-major so one small DMA delivers exactly the
    128-col output block the first attention head pair needs.
  - The V projection and the Q/K projections for nt>=1 run as FILLERS
    inside the attention heads (pack-aligned so head h's PV finds VS ready
    one pack ahead), instead of serializing before head 0.
  - Attention now starts as soon as nt0's Q/K projection lands (~16us).
All matmul operands bf16 (fp32 PSUM accumulation).
"""

import numpy as np
import ml_dtypes

E = 768
T = 1024
B = 8
H = 12
DH = 64
NT = E // 128          # 6 partition-tiles of the embed dim
NJ = T // 128          # 8 partition-tiles of the token dim
SCALE = DH ** -0.5     # folded into the exp() activation's scale operand
VW = H * 128           # V_aug row width: 12 heads x 128 cols (64 data +
                       # ones col; cols 65..127 never written, their PSUM
                       # rows are never read)

# j-tile packs per head: singles for jt<4 (their scores span up to 1024
# cols at natural offsets), {4,5} and {6,7} share one scores tile at
# remapped offsets. entries: (jt, tile_off, width).
PACKS = (
    ((0, 0, 1024),),
    ((1, 128, 896),),
    ((2, 256, 768),),
    ((3, 384, 640),),
    ((4, 0, 512), (5, 512, 384)),
    ((6, 0, 256), (7, 512, 128)),
)

_CACHE = {}


def _build():
    import concourse.bass as bass
    import concourse.tile as tile
    from concourse import bacc, mybir

    f32 = mybir.dt.float32
    bf16 = mybir.dt.bfloat16
    Exp = mybir.ActivationFunctionType.Exp

    nc = bacc.Bacc(
        "TRN2",
        target_bir_lowering=False,
        debug=False,
        enable_asserts=False,
        num_devices=B,
    )

    xt = nc.dram_tensor("xt", [E, T], bf16, kind="ExternalInput").ap()
    # wq/wk host-relaid nt-major: row nt*128+p, col kt*128+c  = W[kt*128+p, nt*128+c]
    wq = nc.dram_tensor("wq", [E, E], bf16, kind="ExternalInput").ap()
    wk = nc.dram_tensor("wk", [E, E], bf16, kind="ExternalInput").ap()
    # wv/wo host-relaid kt-major rows: [p, kt*768 + e] = W[kt*128+p, e]
    wv = nc.dram_tensor("wv", [128, NT * E], bf16, kind="ExternalInput").ap()
    wo = nc.dram_tensor("wo", [128, NT * E], bf16, kind="ExternalInput").ap()
    bqt = nc.dram_tensor("bqt", [128, NT], f32, kind="ExternalInput").ap()
    bkt = nc.dram_tensor("bkt", [128, NT], f32, kind="ExternalInput").ap()
    bvr = nc.dram_tensor("bvr", [1, E], bf16, kind="ExternalInput").ap()
    bor = nc.dram_tensor("bor", [1, E], bf16, kind="ExternalInput").ap()
    tri = nc.dram_tensor("tri", [128, 128], bf16, kind="ExternalInput").ap()
    ones12 = nc.dram_tensor("ones12", [128, NJ * H], bf16, kind="ExternalInput").ap()
    out = nc.dram_tensor("out", [T, E], f32, kind="ExternalOutput").ap()

    with tile.TileContext(nc) as tc:
        with (
            tc.tile_pool(name="const", bufs=1) as cpool,
            tc.tile_pool(name="psb", bufs=3) as ppool,
            tc.tile_pool(name="rsb", bufs=4) as rpool,
            tc.tile_pool(name="rbsb", bufs=4) as rbpool,
            tc.tile_pool(name="fin", bufs=3) as fpool,
            tc.tile_pool(name="pp", bufs=2, space="PSUM") as pp,
            tc.tile_pool(name="sp", bufs=2, space="PSUM") as sp,
            tc.tile_pool(name="op", bufs=2, space="PSUM") as op,
        ):
            XT = cpool.tile([128, NT * T], bf16)     # (kt, i)
            WQ = cpool.tile([128, NT * E], bf16)     # (nt, kt*128+c)  nt-major!
            WK = cpool.tile([128, NT * E], bf16)
            WV = cpool.tile([128, NT * E], bf16)     # (kt, e)         kt-major
            WO = cpool.tile([128, NT * E], bf16)     # (et, n)         et-major
            QS = cpool.tile([128, NT * T], bf16)     # Q' (nt, i)
            KS = cpool.tile([128, NT * T], bf16)
            QS2 = cpool.tile([128, NT * T], bf16)    # partition-swapped copies
            KS2 = cpool.tile([128, NT * T], bf16)
            VS = cpool.tile([128, NJ * VW], bf16)    # (jt, h*128+d); col 64 of
                                                     # each head block = ones
            OS = cpool.tile([128, NT * T], bf16)     # O' (et, i)
            BQ = cpool.tile([128, NT], f32)
            BK = cpool.tile([128, NT], f32)
            BVR = cpool.tile([1, E], bf16)
            BOR = cpool.tile([1, E], bf16)
            TRI = cpool.tile([128, 128], bf16)

            wv3s = wv.rearrange("p (k e) -> p k e", e=E)
            WV3 = WV[:].rearrange("p (k e) -> p k e", e=E)
            wqr = wq.rearrange("(n p) c -> p n c", p=128)
            wkr = wk.rearrange("(n p) c -> p n c", p=128)
            xt3 = xt.rearrange("(k p) i -> p k i", p=128)

            # ---- URGENT input DMAs (issue immediately, small pending set):
            # nt0 blocks of Wq/Wk (unblocks head 0's projection), all of X',
            # V-weight cols for heads 0-1, then the small constants. ----
            nc.sync.dma_start(WQ[:, 0:E], wqr[:, 0])
            nc.sync.dma_start(WK[:, 0:E], wkr[:, 0])
            for kt in range(NT):
                nc.sync.dma_start(XT[:, kt * T : (kt + 1) * T], xt3[:, kt])
            nc.sync.dma_start(WV3[:, :, 0:128], wv3s[:, :, 0:128])
            nc.sync.dma_start(BQ[:], bqt)
            nc.sync.dma_start(BK[:], bkt)
            nc.sync.dma_start(BVR[:], bvr)
            nc.sync.dma_start(BOR[:], bor)
            nc.sync.dma_start(TRI[:], tri)
            # V_aug ones column via one strided DMA
            nc.sync.dma_start(
                VS[:].rearrange("p (j h e) -> p j h e", h=H, e=128)[:, :, :, 64:65],
                ones12.rearrange("p (j h e) -> p j h e", h=H, e=1),
            )
            BVB = cpool.tile([128, E], bf16)
            FINB = cpool.tile([128, E], bf16)
            nc.gpsimd.partition_broadcast(BVB[:], BVR[:])
            nc.gpsimd.partition_broadcast(FINB[:], BOR[:])

            # ---- gated bulk DMA helper: a 1-element DVE copy into the DMA
            # dst corner makes the DMA (WAW) wait until `gate` is produced,
            # keeping the HW queue's round-robin set small early on. ----
            def gated_dma(dst, src, corner, gate):
                nc.vector.tensor_copy(corner, gate)
                nc.sync.dma_start(dst, src)

            def gated_w(Wt, wr, nt, gate):
                gated_dma(
                    Wt[:, nt * E : (nt + 1) * E], wr[:, nt],
                    Wt[0:1, nt * E : nt * E + 1], gate,
                )

            def gated_wv(e0, ew, gate):
                gated_dma(
                    WV3[:, :, e0 : e0 + ew], wv3s[:, :, e0 : e0 + ew],
                    WV[0:1, e0 : e0 + 1], gate,
                )

            # ---- PE warmup: dummy matmuls with no DMA dependency so the
            # HAM activity monitor lifts the 1.2GHz cold gate before real
            # work arrives ----
            DUMW = cpool.tile([128, 128], bf16)
            DUMR = cpool.tile([128, 512], bf16)
            nc.vector.memset(DUMW[:], 1.0)
            nc.vector.memset(DUMR[:], 1.0)

            def dummy(n=512):
                d_ps = pp.tile([128, 512], f32, tag="proj")
                nc.tensor.matmul(
                    d_ps[:, :n], lhsT=DUMW[:], rhs=DUMR[:, :n], start=True, stop=True
                )

            def dummy_sc(n=512):
                d_ps = sp.tile([128, 1024], f32, tag="scores")
                nc.tensor.matmul(
                    d_ps[:, :n], lhsT=DUMW[:], rhs=DUMR[:, :n], start=True, stop=True
                )

            for _ in range(26):
                dummy()

            # ---- V projection for one (j-tile, col-chunk): cols e0..e0+ew
            # of V (= heads e0//64 .. (e0+ew)//64), evicted into VS. ----
            def v_group(jt, e0, ew):
                h0, nh = e0 // 64, ew // 64
                ps = pp.tile([128, 512], f32, tag="proj", name=f"vg{jt}_{e0}")
                for kt in range(NT):
                    nc.tensor.matmul(
                        ps[:, :ew],
                        lhsT=XT[:, kt * T + jt * 128 : kt * T + jt * 128 + 128],
                        rhs=WV[:, kt * E + e0 : kt * E + e0 + ew],
                        start=(kt == 0),
                        stop=(kt == NT - 1),
                        skip_group_check=True,
                    )
                # heads are 64 cols each; VS blocks are 128 cols per head.
                # e0/ew are multiples of 128 -> whole head blocks.
                dst = (
                    VS[:, jt * VW + (e0 // 64) * 128 : jt * VW + ((e0 + ew) // 64) * 128]
                    .rearrange("p (h e) -> p h e", e=128)[:, :, 0:64]
                )
                nc.vector.tensor_add(
                    dst,
                    ps[:, :ew].rearrange("p (h d) -> p h d", d=64),
                    BVB[:, e0 : e0 + ew].rearrange("p (h d) -> p h d", d=64),
                )

            # ---- Q'/K' projection for one 128-row block nt (2 heads);
            # after both i-chunks, SBUF->SBUF DMAs build the partition-
            # swapped copies ----
            def qk_group(nt, g):
                W, Bb, DST, DST2 = (
                    (WQ, BQ, QS, QS2) if g < 2 else (WK, BK, KS, KS2)
                )
                ic = g % 2
                ps = pp.tile([128, 512], f32, tag="proj", name=f"qkg{nt}_{g}")
                for kt in range(NT):
                    nc.tensor.matmul(
                        ps[:],
                        lhsT=W[:, nt * E + kt * 128 : nt * E + kt * 128 + 128],
                        rhs=XT[:, kt * T + ic * 512 : kt * T + ic * 512 + 512],
                        start=(kt == 0),
                        stop=(kt == NT - 1),
                        skip_group_check=True,
                    )
                nc.vector.tensor_scalar_add(
                    DST[:, nt * T + ic * 512 : nt * T + ic * 512 + 512],
                    ps[:],
                    Bb[:, nt : nt + 1],
                )
                if ic == 1:  # both chunks done -> build the mirror copy
                    cols = slice(nt * T, nt * T + T)
                    nc.sync.dma_start(DST2[64:128, cols], DST[0:64, cols])
                    nc.sync.dma_start(DST2[0:64, cols], DST[64:128, cols])

            def normalize(o_ps, h, ic):
                # softmax denominators live in row 64 (the V_aug ones column).
                nt, po = h // 2, (h % 2) * 64
                dn = rpool.tile([1, 512], f32, tag="denom")
                nc.vector.tensor_copy(dn[:], o_ps[64:65, :])
                r = rpool.tile([1, 512], f32, tag="recip")
                nc.vector.reciprocal_approx_fast(r[:], dn[:])
                rb = rbpool.tile([64, 512], f32, tag="recipb")
                nc.gpsimd.partition_broadcast(rb[:], r[:])
                nc.vector.tensor_mul(
                    OS[po : po + 64, nt * T + ic * 512 : nt * T + ic * 512 + 512],
                    o_ps[0:64, :],
                    rb[:],
                )

            # ---- O-projection accumulation chunk ----
            def oproj_chunk(it, n0, nw, ps, ets, first, last):
                for et in ets:
                    nc.tensor.matmul(
                        ps[:, :nw],
                        lhsT=OS[:, et * T + it * 128 : et * T + it * 128 + 128],
                        rhs=WO[:, et * E + n0 : et * E + n0 + nw],
                        start=(first and et == ets[0]),
                        stop=(last and et == ets[-1]),
                        skip_group_check=True,
                    )

            # ---- attention for one head, KQ orientation (see v1 notes) ----
            pre = []

            def head(h, fillers=None):
                fillers = fillers or {}
                nt, po = h // 2, (h % 2) * 64
                pm = po ^ 64  # mirror partitions
                o_ps0 = op.tile([128, 512], f32, tag="oaug")
                o_ps1 = op.tile([128, 512], f32, tag="oaug")

                def pv(pi, p2):
                    for jt, off, w in PACKS[pi]:
                        d0 = jt * 128
                        lhsV = VS[:, jt * VW + h * 128 : jt * VW + h * 128 + 128]
                        if jt < 4:
                            nc.tensor.matmul(
                                o_ps0[:, d0:512],
                                lhsT=lhsV,
                                rhs=p2[:, d0:512],
                                start=(jt == 0),
                                stop=(jt == 3),
                                skip_group_check=True,
                            )
                            nc.tensor.matmul(
                                o_ps1[:, 0:512],
                                lhsT=lhsV,
                                rhs=p2[:, 512:1024],
                                start=(jt == 0),
                                stop=(jt == NJ - 1),
                                skip_group_check=True,
                            )
                        else:
                            nc.tensor.matmul(
                                o_ps1[:, d0 - 512 : 512],
                                lhsT=lhsV,
                                rhs=p2[:, off : off + w],
                                start=(jt == 0),
                                stop=(jt == NJ - 1),
                                skip_group_check=True,
                            )
                    if pi == 3:
                        normalize(o_ps0, h, 0)

                prev = None
                for pi, pack in enumerate(PACKS):
                    s2 = sp.tile([128, 1024], f32, tag="scores")
                    p2 = ppool.tile([128, 1024], bf16, tag="probs")
                    # S matmuls: the two matmuls of each pack read
                    # alternating (original/mirrored) copies -> adjacent
                    # issues on disjoint row groups -> concurrent
                    if len(pack) == 1:
                        jt, off, w = pack[0]
                        d0 = jt * 128
                        nc.tensor.matmul(
                            s2[:, d0:512],
                            lhsT=KS[po : po + 64, nt * T + d0 : nt * T + d0 + 128],
                            rhs=QS[po : po + 64, nt * T + d0 : nt * T + 512],
                            start=True,
                            stop=True,
                        )
                        nc.tensor.matmul(
                            s2[:, 512:1024],
                            lhsT=KS2[pm : pm + 64, nt * T + d0 : nt * T + d0 + 128],
                            rhs=QS2[pm : pm + 64, nt * T + 512 : nt * T + 1024],
                            start=True,
                            stop=True,
                        )
                    else:
                        (jta, offa, wa), (jtb, offb, wb) = pack
                        da, db_ = jta * 128, jtb * 128
                        # second matmul on the mirrored row group ONLY if its
                        # columns land in the other PSUM bank
                        mirror = offa < 512 <= offb
                        K2, Q2, p2_ = (KS2, QS2, pm) if mirror else (KS, QS, po)
                        nc.tensor.matmul(
                            s2[:, offa : offa + wa],
                            lhsT=KS[po : po + 64, nt * T + da : nt * T + da + 128],
                            rhs=QS[po : po + 64, nt * T + da : nt * T + da + wa],
                            start=True,
                            stop=True,
                        )
                        nc.tensor.matmul(
                            s2[:, offb : offb + wb],
                            lhsT=K2[p2_ : p2_ + 64, nt * T + db_ : nt * T + db_ + 128],
                            rhs=Q2[p2_ : p2_ + 64, nt * T + db_ : nt * T + db_ + wb],
                            start=True,
                            stop=True,
                        )
                    for f in fillers.get(pi, ()):
                        f()
                    lo = pack[0][1]
                    hi = pack[-1][1] + pack[-1][2]
                    nc.scalar.activation(p2[:, lo:hi], s2[:, lo:hi], Exp, scale=SCALE)
                    for jt, off, w in pack:
                        nc.vector.tensor_mul(
                            p2[:, off : off + 128], p2[:, off : off + 128], TRI[:]
                        )
                    if prev is not None:
                        pv(*prev)
                    prev = (pi, p2)
                pv(*prev)
                normalize(o_ps1, h, 1)

            def pre_acc(n0, nw):
                def f():
                    f0 = pp.tile([128, 512], f32, tag="proj", name=f"pre{n0}")
                    oproj_chunk(0, n0, nw, f0, range(5), True, False)
                    pre.append((f0, n0, nw))
                return f

            def qk(nt, g):
                return lambda: qk_group(nt, g)

            def vg(jt, e0, ew):
                return lambda: v_group(jt, e0, ew)

            # ---- nt0 Q/K projection up-front (DMA-paced by XT landing) ----
            for g in range(4):
                qk_group(0, g)
            # bulk DMAs gated on nt0 projection progress
            gated_w(WQ, wqr, 1, QS[0:1, 512:513])
            gated_w(WK, wkr, 1, QS[0:1, 512:513])
            gated_wv(128, 128, QS[0:1, 512:513])
            gated_w(WQ, wqr, 2, KS[0:1, 512:513])
            gated_w(WK, wkr, 2, KS[0:1, 512:513])
            gated_wv(256, 256, KS[0:1, 512:513])

            # ---- filler schedule: V projection chunks + later-nt Q/K
            # projections + O-proj pre-accumulation, pack-aligned ----
            FILL = {
                0: {0: [vg(0, 0, 128)], 1: [vg(1, 0, 128)], 2: [vg(2, 0, 128)],
                    3: [vg(3, 0, 128)], 4: [vg(4, 0, 128), vg(5, 0, 128)],
                    5: [vg(6, 0, 128), vg(7, 0, 128)]},
                1: {0: [vg(0, 128, 128), vg(1, 128, 128)],
                    1: [vg(2, 128, 128), vg(3, 128, 128)],
                    2: [vg(4, 128, 128), vg(5, 128, 128)],
                    3: [vg(6, 128, 128), vg(7, 128, 128)],
                    4: [qk(1, 0), qk(1, 2)], 5: [qk(1, 1), qk(1, 3)]},
                2: {0: [qk(2, 0)], 1: [qk(2, 1)],
                    2: [vg(0, 256, 256), vg(1, 256, 256)],
                    3: [vg(2, 256, 256), vg(3, 256, 256)],
                    4: [vg(4, 256, 256), vg(5, 256, 256)],
                    5: [vg(6, 256, 256), vg(7, 256, 256)]},
                3: {0: [qk(2, 2)], 1: [qk(2, 3)], 2: [qk(3, 0)], 3: [qk(3, 1)],
                    4: [vg(0, 512, 256), vg(1, 512, 256)],
                    5: [vg(2, 512, 256), vg(3, 512, 256)]},
                4: {0: [vg(4, 512, 256), vg(5, 512, 256)],
                    1: [vg(6, 512, 256), vg(7, 512, 256)],
                    2: [qk(3, 2)], 3: [qk(3, 3)], 4: [qk(4, 0)], 5: [qk(4, 1)]},
                5: {1: [qk(4, 2)], 3: [qk(4, 3)], 5: [dummy]},
                6: {1: [qk(5, 0)], 3: [qk(5, 1)], 5: [dummy]},
                7: {1: [qk(5, 2)], 3: [qk(5, 3)], 5: [dummy]},
                8: {1: [dummy], 3: [dummy], 5: [dummy]},
                9: {1: [dummy], 3: [dummy], 5: [dummy]},
                10: {1: [dummy], 3: [dummy], 5: [dummy]},
                11: {0: [dummy], 1: [dummy], 2: [dummy], 3: [dummy],
                     4: [pre_acc(0, 512)], 5: [pre_acc(512, 256)]},
            }
            for h in range(H):
                head(h, FILL.get(h))
                if h == 0:  # OS head0-ic0 now exists -> release late bulk
                    gated_w(WQ, wqr, 3, OS[0:1, 0:1])
                    gated_w(WK, wkr, 3, OS[0:1, 0:1])
                    gated_wv(512, 256, OS[0:1, 0:1])
                    gated_dma(WO[:], wo, WO[0:1, 0:1], OS[0:1, 0:1])
                elif h == 1:
                    gated_w(WQ, wqr, 4, OS[64:65, 0:1])
                    gated_w(WK, wkr, 4, OS[64:65, 0:1])
                elif h == 2:
                    gated_w(WQ, wqr, 5, OS[0:1, T : T + 1])
                    gated_w(WK, wkr, 5, OS[0:1, T : T + 1])

            # bridge the final-normalize window (scores ring: the proj ring
            # holds the live pre-accumulated it=0 tiles)
            for _ in range(6):
                dummy_sc()

            # ---- output projection ----
            fin0 = fpool.tile([128, E], f32, tag="fin")
            for f_ps, n0, nw in pre:
                oproj_chunk(0, n0, nw, f_ps, [NT - 1], False, True)
                nc.vector.tensor_add(
                    fin0[:, n0 : n0 + nw], f_ps[:, :nw], FINB[:, n0 : n0 + nw]
                )
                nc.sync.dma_start(out[0:128, n0 : n0 + nw], fin0[:, n0 : n0 + nw])
            for it in range(1, NJ):
                fin = fpool.tile([128, E], f32, tag="fin")
                for n0, nw in ((0, 512), (512, 256)):
                    f_ps = pp.tile([128, 512], f32, tag="proj")
                    oproj_chunk(it, n0, nw, f_ps, range(NT), True, True)
                    nc.vector.tensor_add(
                        fin[:, n0 : n0 + nw], f_ps[:, :nw], FINB[:, n0 : n0 + nw]
                    )
                    nc.sync.dma_start(
                        out[it * 128 : (it + 1) * 128, n0 : n0 + nw],
                        fin[:, n0 : n0 + nw],
                    )

    nc.compile()
    return nc


def _get_nc():
    if "nc" not in _CACHE:
        _CACHE["nc"] = _build()
    return _CACHE["nc"]


def _make_in_maps(inputs):
    bf = ml_dtypes.bfloat16
    x = np.asarray(inputs["x"], np.float32)

    def nt_major(w):
        # row nt*128+p, col kt*128+c = W[kt*128+p, nt*128+c]
        w4 = np.asarray(w, np.float32).reshape(NT, 128, NT, 128)  # [kt,p,nt,c]
        return np.ascontiguousarray(w4.transpose(2, 1, 0, 3).reshape(E, E)).astype(bf)

    def kt_major(w):
        # [p, kt*768+e] = W[kt*128+p, e]
        w3 = np.asarray(w, np.float32).reshape(NT, 128, E)  # [kt,p,e]
        return np.ascontiguousarray(w3.transpose(1, 0, 2).reshape(128, NT * E)).astype(bf)

    shared = {
        "wq": nt_major(inputs["Wq"]),
        "wk": nt_major(inputs["Wk"]),
        "wv": kt_major(inputs["Wv"]),
        "wo": kt_major(inputs["Wo"]),
        "bqt": np.ascontiguousarray(
            np.asarray(inputs["bq"], np.float32).reshape(NT, 128).T
        ),
        "bkt": np.ascontiguousarray(
            np.asarray(inputs["bk"], np.float32).reshape(NT, 128).T
        ),
        "bvr": np.asarray(inputs["bv"], np.float32).reshape(1, E).astype(bf),
        "bor": np.asarray(inputs["bo"], np.float32).reshape(1, E).astype(bf),
        "tri": np.triu(np.ones((128, 128), np.float32)).astype(bf),
        "ones12": np.ones((128, NJ * H), np.float32).astype(bf),
    }
    return [dict(shared, xt=x[b].T.astype(bf)) for b in range(B)]


def _run(inputs, trace=False):
    from concourse import bass_utils

    nc = _get_nc()
    res = bass_utils.run_bass_kernel_spmd(
        nc, _make_in_maps(inputs), core_ids=list(range(B)), trace=trace
    )
    out = np.stack([np.asarray(res.results[c]["out"]) for c in range(B)])
    return out, res


def kernel(**inputs) -> np.ndarray:
    out, _ = _run(inputs, trace=False)
    return out


# revision 11
# speedup vs baseline: 1.2595x; 1.2595x over previous
"""CLIP causal attention (B=8, T=1024, E=768, H=12) on 8 TRN2 NeuronCores.

Strategy: pure data-parallel over batch — core b handles x[b] end to end,
no collectives. All compute in transposed space (embed on partitions):

  X' = x_b^T                       [768, 1024]  (host pre-transposed, bf16)
  Q' = Wq^T @ X' (+bq)             [768, 1024]  lhsT = Wq as stored
  K' = Wk^T @ X' (+bk)             [768, 1024]
  V  = X'^T @ Wv (+bv)             [1024, 768]  lhsT = X' blocks (j on partitions)
  per head h (KQ orientation, j on partitions, i free):
     S'[j,i] = K'_h[:,jblk]^T @ Q'_h          (K=64)
       QS2/KS2 are partition-SWAPPED copies of Q'/K' (halves exchanged via
       SBUF->SBUF DMA): each pack's two S matmuls read alternating copies ->
       adjacent issues target disjoint PE row groups and run CONCURRENTLY.
     P' = exp(S' * 1/8)  (no max-subtraction: |S'/8| <= ~7, exact-safe)
     causal: skip fully-masked blocks, restrict to valid cols, tri-mask diag
     PV matmuls run one pack late (software pipeline): each pack's exp()
       hides under the next pack's S matmuls.
     O_aug[d,i] = sum_j Vaug_h[j,d]^T @ P'    (Vaug ones column -> row 64 =
                                               softmax denominator)
     O'_h = O_aug[0:64] * broadcast(1/denom)
  out = (O'^T @ Wo) + bo           [1024, 768]

v2 restructure (startup was 52us of DMA-starved projections in v1):
  - The single HW DGE queue round-robins packets across ALL pending DMAs,
    so nothing lands until the whole input stream is done.  Input DMAs are
    now split into an URGENT set (Wq/Wk nt0 blocks, X', V-weights for
    heads 0-1) issued immediately, and a BULK set gated behind compute
    progress via 1-element DVE corner-copies (WAW dep on the DMA dst makes
    the sync engine hold the issue until the gate tile is produced).
  - Wq/Wk are host-relaid nt-major so one small DMA delivers exactly the
    128-col output block the first attention head pair needs.
  - The V projection and the Q/K projections for nt>=1 run as FILLERS
    inside the attention heads (pack-aligned so head h's PV finds VS ready
    one pack ahead), instead of serializing before head 0.
  - Attention now starts as soon as nt0's Q/K projection lands (~16us).
All matmul operands bf16 (fp32 PSUM accumulation).
"""

import numpy as np
import ml_dtypes

E = 768
T = 1024
B = 8
H = 12
DH = 64
NT = E // 128          # 6 partition-tiles of the embed dim
NJ = T // 128          # 8 partition-tiles of the token dim
SCALE = DH ** -0.5     # folded into the exp() activation's scale operand
VW = H * 128           # V_aug row width: 12 heads x 128 cols (64 data +
                       # ones col; cols 65..127 never written, their PSUM
                       # rows are never read)

# j-tile packs per head: singles for jt<4 (their scores span up to 1024
# cols at natural offsets), {4,5} and {6,7} share one scores tile at
# remapped offsets. entries: (jt, tile_off, width).
PACKS = (
    ((0, 0, 1024),),
    ((1, 128, 896),),
    ((2, 256, 768),),
    ((3, 384, 640),),
    ((4, 0, 512), (5, 512, 384)),
    ((6, 0, 256), (7, 512, 128)),
)

_CACHE = {}


def _build():
    import concourse.bass as bass
    import concourse.tile as tile
    from concourse import bacc, mybir

    f32 = mybir.dt.float32
    bf16 = mybir.dt.bfloat16
    Exp = mybir.ActivationFunctionType.Exp

    nc = bacc.Bacc(
        "TRN2",
        target_bir_lowering=False,
        debug=False,
        enable_asserts=False,
        num_devices=B,
    )

    xt = nc.dram_tensor("xt", [E, T], bf16, kind="ExternalInput").ap()
    # wq/wk host-relaid nt-major: row nt*128+p, col kt*128+c  = W[kt*128+p, nt*128+c]
    wq = nc.dram_tensor("wq", [E, E], bf16, kind="ExternalInput").ap()
    wk = nc.dram_tensor("wk", [E, E], bf16, kind="ExternalInput").ap()
    # wv chunk-major: 4 col-chunks of V, each contiguous as [p, kt*ew + c]
    # (chunk col offset, e0, ew): (0,0,128) (768,128,128) (1536,256,256) (3072,512,256)
    wv = nc.dram_tensor("wv", [128, NT * E], bf16, kind="ExternalInput").ap()
    # wo et-major rows: [p, et*768 + n] = Wo[et*128+p, n]
    wo = nc.dram_tensor("wo", [128, NT * E], bf16, kind="ExternalInput").ap()
    bqt = nc.dram_tensor("bqt", [128, NT], f32, kind="ExternalInput").ap()
    bkt = nc.dram_tensor("bkt", [128, NT], f32, kind="ExternalInput").ap()
    bvr = nc.dram_tensor("bvr", [1, E], bf16, kind="ExternalInput").ap()
    bor = nc.dram_tensor("bor", [1, E], bf16, kind="ExternalInput").ap()
    tri = nc.dram_tensor("tri", [128, 128], bf16, kind="ExternalInput").ap()
    out = nc.dram_tensor("out", [T, E], f32, kind="ExternalOutput").ap()

    VCHUNKS = ((0, 0, 128), (768, 128, 128), (1536, 256, 256), (3072, 512, 256))

    with tile.TileContext(nc) as tc:
        with (
            tc.tile_pool(name="const", bufs=1) as cpool,
            tc.tile_pool(name="psb", bufs=3) as ppool,
            tc.tile_pool(name="rsb", bufs=4) as rpool,
            tc.tile_pool(name="rbsb", bufs=4) as rbpool,
            tc.tile_pool(name="fin", bufs=3) as fpool,
            tc.tile_pool(name="pp", bufs=2, space="PSUM") as pp,
            tc.tile_pool(name="sp", bufs=2, space="PSUM") as sp,
            tc.tile_pool(name="op", bufs=2, space="PSUM") as op,
        ):
            XT = cpool.tile([128, NT * T], bf16)     # (kt, i)
            WQ = cpool.tile([128, NT * E], bf16)     # (nt, kt*128+c)  nt-major!
            WK = cpool.tile([128, NT * E], bf16)
            WV = cpool.tile([128, NT * E], bf16)     # (kt, e)         kt-major
            WO = cpool.tile([128, NT * E], bf16)     # (et, n)         et-major
            QS = cpool.tile([128, NT * T], bf16)     # Q' (nt, i)
            KS = cpool.tile([128, NT * T], bf16)
            QS2 = cpool.tile([128, NT * T], bf16)    # partition-swapped copies
            KS2 = cpool.tile([128, NT * T], bf16)
            VS = cpool.tile([128, NJ * VW], bf16)    # (jt, h*128+d); col 64 of
                                                     # each head block = ones
            OS = cpool.tile([128, NT * T], bf16)     # O' (et, i)
            BQ = cpool.tile([128, NT], f32)
            BK = cpool.tile([128, NT], f32)
            BVR = cpool.tile([1, E], bf16)
            BOR = cpool.tile([1, E], bf16)
            TRI = cpool.tile([128, 128], bf16)

            wqr = wq.rearrange("(n p) c -> p n c", p=128)
            wkr = wk.rearrange("(n p) c -> p n c", p=128)
            xt3 = xt.rearrange("(k p) i -> p k i", p=128)

            # ---- URGENT input DMAs (issue immediately, small pending set):
            # nt0 blocks of Wq/Wk (unblocks head 0's projection), all of X',
            # V-weight cols for heads 0-1, then the small constants. ----
            nc.sync.dma_start(WQ[:, 0:E], wqr[:, 0])
            nc.sync.dma_start(WK[:, 0:E], wkr[:, 0])
            for kt in range(NT):
                nc.sync.dma_start(XT[:, kt * T : (kt + 1) * T], xt3[:, kt])
            co, e0, ew = VCHUNKS[0]
            nc.sync.dma_start(WV[:, co : co + NT * ew], wv[:, co : co + NT * ew])
            nc.sync.dma_start(BQ[:], bqt)
            nc.sync.dma_start(BK[:], bkt)
            nc.sync.dma_start(BVR[:], bvr)
            nc.sync.dma_start(BOR[:], bor)
            nc.sync.dma_start(TRI[:], tri)
            BVB = cpool.tile([128, E], bf16)
            FINB = cpool.tile([128, E], bf16)
            nc.gpsimd.partition_broadcast(BVB[:], BVR[:])
            nc.gpsimd.partition_broadcast(FINB[:], BOR[:])
            # V_aug ones column: one strided DVE memset (cheap vs the
            # 6.8us of HW-DGE descriptor generation the DMA version costs)
            nc.vector.memset(
                VS[:].rearrange("p (j h e) -> p j h e", h=H, e=128)[:, :, :, 64:65],
                1.0,
            )

            # ---- gated bulk DMA helper: a 1-element DVE copy into the DMA
            # dst corner makes the DMA (WAW) wait until `gate` is produced,
            # keeping the HW queue's round-robin set small early on. ----
            def gated_dma(dst, src, corner, gate):
                nc.vector.tensor_copy(corner, gate)
                nc.sync.dma_start(dst, src)

            def gated_w(Wt, wr, nt, gate):
                gated_dma(
                    Wt[:, nt * E : (nt + 1) * E], wr[:, nt],
                    Wt[0:1, nt * E : nt * E + 1], gate,
                )

            def gated_wv(ci, gate):
                co, e0, ew = VCHUNKS[ci]
                gated_dma(
                    WV[:, co : co + NT * ew], wv[:, co : co + NT * ew],
                    WV[0:1, co : co + 1], gate,
                )

            # ---- PE warmup: dummy matmuls with no DMA dependency so the
            # HAM activity monitor lifts the 1.2GHz cold gate before real
            # work arrives ----
            DUMW = cpool.tile([128, 128], bf16)
            DUMR = cpool.tile([128, 512], bf16)
            nc.vector.memset(DUMW[:], 1.0)
            nc.vector.memset(DUMR[:], 1.0)

            def dummy(n=512):
                d_ps = pp.tile([128, 512], f32, tag="proj")
                nc.tensor.matmul(
                    d_ps[:, :n], lhsT=DUMW[:], rhs=DUMR[:, :n], start=True, stop=True
                )

            def dummy_sc(n=512):
                d_ps = sp.tile([128, 1024], f32, tag="scores")
                nc.tensor.matmul(
                    d_ps[:, :n], lhsT=DUMW[:], rhs=DUMR[:, :n], start=True, stop=True
                )

            for _ in range(26):
                dummy()

            # ---- V projection for one (j-tile, col-chunk ci): cols
            # e0..e0+ew of V (heads e0//64 ..), evicted into VS. ----
            def v_group(jt, ci):
                co, e0, ew = VCHUNKS[ci]
                ps = pp.tile([128, 512], f32, tag="proj", name=f"vg{jt}_{e0}")
                for kt in range(NT):
                    nc.tensor.matmul(
                        ps[:, :ew],
                        lhsT=XT[:, kt * T + jt * 128 : kt * T + jt * 128 + 128],
                        rhs=WV[:, co + kt * ew : co + kt * ew + ew],
                        start=(kt == 0),
                        stop=(kt == NT - 1),
                        skip_group_check=True,
                    )
                # heads are 64 cols each; VS blocks are 128 cols per head.
                # e0/ew are multiples of 128 -> whole head blocks.
                dst = (
                    VS[:, jt * VW + (e0 // 64) * 128 : jt * VW + ((e0 + ew) // 64) * 128]
                    .rearrange("p (h e) -> p h e", e=128)[:, :, 0:64]
                )
                nc.vector.tensor_add(
                    dst,
                    ps[:, :ew].rearrange("p (h d) -> p h d", d=64),
                    BVB[:, e0 : e0 + ew].rearrange("p (h d) -> p h d", d=64),
                )

            # ---- Q'/K' projection for one 128-row block nt (2 heads);
            # after both i-chunks, SBUF->SBUF DMAs build the partition-
            # swapped copies ----
            def qk_group(nt, g):
                W, Bb, DST, DST2 = (
                    (WQ, BQ, QS, QS2) if g < 2 else (WK, BK, KS, KS2)
                )
                ic = g % 2
                ps = pp.tile([128, 512], f32, tag="proj", name=f"qkg{nt}_{g}")
                for kt in range(NT):
                    nc.tensor.matmul(
                        ps[:],
                        lhsT=W[:, nt * E + kt * 128 : nt * E + kt * 128 + 128],
                        rhs=XT[:, kt * T + ic * 512 : kt * T + ic * 512 + 512],
                        start=(kt == 0),
                        stop=(kt == NT - 1),
                        skip_group_check=True,
                    )
                nc.vector.tensor_scalar_add(
                    DST[:, nt * T + ic * 512 : nt * T + ic * 512 + 512],
                    ps[:],
                    Bb[:, nt : nt + 1],
                )
                if ic == 1:  # both chunks done -> build the mirror copy.
                    # Issued on the ACT HW-DGE queue so they don't round-robin
                    # with the bulk weight stream on the sync queue.  QS2 is
                    # only ever read at cols [512:1024) -> half mirror.
                    if DST is QS:
                        cols = slice(nt * T + 512, nt * T + T)
                    else:
                        cols = slice(nt * T, nt * T + T)
                    nc.scalar.dma_start(DST2[64:128, cols], DST[0:64, cols])
                    nc.scalar.dma_start(DST2[0:64, cols], DST[64:128, cols])

            def normalize(o_ps, h, ic):
                # softmax denominators live in row 64 (the V_aug ones column).
                nt, po = h // 2, (h % 2) * 64
                dn = rpool.tile([1, 512], f32, tag="denom")
                nc.vector.tensor_copy(dn[:], o_ps[64:65, :])
                r = rpool.tile([1, 512], f32, tag="recip")
                nc.vector.reciprocal_approx_fast(r[:], dn[:])
                rb = rbpool.tile([64, 512], f32, tag="recipb")
                nc.gpsimd.partition_broadcast(rb[:], r[:])
                nc.vector.tensor_mul(
                    OS[po : po + 64, nt * T + ic * 512 : nt * T + ic * 512 + 512],
                    o_ps[0:64, :],
                    rb[:],
                )

            # ---- O-projection accumulation chunk ----
            def oproj_chunk(it, n0, nw, ps, ets, first, last):
                for et in ets:
                    nc.tensor.matmul(
                        ps[:, :nw],
                        lhsT=OS[:, et * T + it * 128 : et * T + it * 128 + 128],
                        rhs=WO[:, et * E + n0 : et * E + n0 + nw],
                        start=(first and et == ets[0]),
                        stop=(last and et == ets[-1]),
                        skip_group_check=True,
                    )

            # ---- attention for one head, KQ orientation (see v1 notes) ----
            pre = []

            def head(h, fillers=None):
                fillers = fillers or {}
                nt, po = h // 2, (h % 2) * 64
                pm = po ^ 64  # mirror partitions
                o_ps0 = op.tile([128, 512], f32, tag="oaug")
                o_ps1 = op.tile([128, 512], f32, tag="oaug")

                def pv(pi, p2):
                    for jt, off, w in PACKS[pi]:
                        d0 = jt * 128
                        lhsV = VS[:, jt * VW + h * 128 : jt * VW + h * 128 + 128]
                        if jt < 4:
                            nc.tensor.matmul(
                                o_ps0[:, d0:512],
                                lhsT=lhsV,
                                rhs=p2[:, d0:512],
                                start=(jt == 0),
                                stop=(jt == 3),
                                skip_group_check=True,
                            )
                            nc.tensor.matmul(
                                o_ps1[:, 0:512],
                                lhsT=lhsV,
                                rhs=p2[:, 512:1024],
                                start=(jt == 0),
                                stop=(jt == NJ - 1),
                                skip_group_check=True,
                            )
                        else:
                            nc.tensor.matmul(
                                o_ps1[:, d0 - 512 : 512],
                                lhsT=lhsV,
                                rhs=p2[:, off : off + w],
                                start=(jt == 0),
                                stop=(jt == NJ - 1),
                                skip_group_check=True,
                            )
                    if pi == 3:
                        normalize(o_ps0, h, 0)

                prev = None
                for pi, pack in enumerate(PACKS):
                    s2 = sp.tile([128, 1024], f32, tag="scores")
                    p2 = ppool.tile([128, 1024], bf16, tag="probs")
                    # S matmuls: the two matmuls of each pack read
                    # alternating (original/mirrored) copies -> adjacent
                    # issues on disjoint row groups -> concurrent
                    if len(pack) == 1:
                        jt, off, w = pack[0]
                        d0 = jt * 128
                        nc.tensor.matmul(
                            s2[:, d0:512],
                            lhsT=KS[po : po + 64, nt * T + d0 : nt * T + d0 + 128],
                            rhs=QS[po : po + 64, nt * T + d0 : nt * T + 512],
                            start=True,
                            stop=True,
                        )
                        nc.tensor.matmul(
                            s2[:, 512:1024],
                            lhsT=KS2[pm : pm + 64, nt * T + d0 : nt * T + d0 + 128],
                            rhs=QS2[pm : pm + 64, nt * T + 512 : nt * T + 1024],
                            start=True,
                            stop=True,
                        )
                    else:
                        (jta, offa, wa), (jtb, offb, wb) = pack
                        da, db_ = jta * 128, jtb * 128
                        # second matmul on the mirrored row group ONLY if its
                        # columns land in the other PSUM bank
                        mirror = offa < 512 <= offb
                        K2, Q2, p2_ = (KS2, QS2, pm) if mirror else (KS, QS, po)
                        nc.tensor.matmul(
                            s2[:, offa : offa + wa],
                            lhsT=KS[po : po + 64, nt * T + da : nt * T + da + 128],
                            rhs=QS[po : po + 64, nt * T + da : nt * T + da + wa],
                            start=True,
                            stop=True,
                        )
                        nc.tensor.matmul(
                            s2[:, offb : offb + wb],
                            lhsT=K2[p2_ : p2_ + 64, nt * T + db_ : nt * T + db_ + 128],
                            rhs=Q2[p2_ : p2_ + 64, nt * T + db_ : nt * T + db_ + wb],
                            start=True,
                            stop=True,
                        )
                    for f in fillers.get(pi, ()):
                        f()
                    lo = pack[0][1]
                    hi = pack[-1][1] + pack[-1][2]
                    nc.scalar.activation(p2[:, lo:hi], s2[:, lo:hi], Exp, scale=SCALE)
                    for jt, off, w in pack:
                        nc.vector.tensor_mul(
                            p2[:, off : off + 128], p2[:, off : off + 128], TRI[:]
                        )
                    if prev is not None:
                        pv(*prev)
                    prev = (pi, p2)
                pv(*prev)
                normalize(o_ps1, h, 1)

            def pre_acc(n0, nw):
                def f():
                    f0 = pp.tile([128, 512], f32, tag="proj", name=f"pre{n0}")
                    oproj_chunk(0, n0, nw, f0, range(5), True, False)
                    pre.append((f0, n0, nw))
                return f

            def qk(nt, g):
                return lambda: qk_group(nt, g)

            def vg(jt, ci):
                return lambda: v_group(jt, ci)

            # ---- nt0 Q/K projection up-front (DMA-paced by XT landing) ----
            for g in range(4):
                qk_group(0, g)
            # bulk DMAs gated on nt0 projection progress
            gated_w(WQ, wqr, 1, QS[0:1, 512:513])
            gated_w(WK, wkr, 1, QS[0:1, 512:513])
            gated_wv(1, QS[0:1, 512:513])
            gated_w(WQ, wqr, 2, KS[0:1, 512:513])
            gated_w(WK, wkr, 2, KS[0:1, 512:513])
            gated_wv(2, KS[0:1, 512:513])

            # ---- filler schedule: V projection chunks + later-nt Q/K
            # projections + O-proj pre-accumulation, pack-aligned.  Work
            # items are spread so no head goes PE-sparse (HAM re-throttle)
            # while meeting each item's deadline (qk.nt by head 2nt's S,
            # vg chunk by its first consumer head's PV). ----
            FILL = {
                0: {0: [vg(0, 0)], 1: [vg(1, 0)], 2: [vg(2, 0)],
                    3: [vg(3, 0)], 4: [vg(4, 0), vg(5, 0)],
                    5: [vg(6, 0), vg(7, 0)]},
                1: {0: [vg(0, 1), vg(1, 1)], 1: [vg(2, 1), vg(3, 1)],
                    2: [vg(4, 1), vg(5, 1)], 3: [vg(6, 1), vg(7, 1)],
                    4: [qk(1, 0), qk(1, 2)], 5: [qk(1, 1), qk(1, 3)]},
                2: {0: [qk(2, 0)], 1: [vg(0, 2), vg(1, 2)],
                    2: [qk(2, 1)], 3: [vg(2, 2), vg(3, 2)],
                    4: [vg(4, 2), vg(5, 2)], 5: [dummy]},
                3: {0: [qk(2, 2)], 1: [vg(6, 2), vg(7, 2)],
                    2: [qk(2, 3)], 3: [dummy], 4: [dummy], 5: [dummy]},
                4: {0: [qk(3, 0)], 1: [vg(0, 3), vg(1, 3)],
                    2: [qk(3, 1)], 3: [vg(2, 3), vg(3, 3)],
                    4: [qk(3, 2)], 5: [dummy]},
                5: {0: [qk(3, 3)], 1: [vg(4, 3), vg(5, 3)],
                    2: [qk(4, 0)], 3: [vg(6, 3), vg(7, 3)],
                    4: [qk(4, 1)], 5: [dummy]},
                6: {0: [qk(4, 2)], 2: [qk(4, 3)], 4: [dummy]},
                7: {0: [qk(5, 0)], 2: [qk(5, 1)], 4: [dummy]},
                8: {0: [qk(5, 2)], 2: [qk(5, 3)], 4: [dummy]},
                9: {0: [dummy], 2: [dummy], 4: [dummy]},
                10: {0: [dummy], 1: [dummy], 2: [dummy], 3: [dummy],
                     4: [pre_acc(0, 512)], 5: [pre_acc(512, 256)]},
                # h11: no fillers — the pp ring is pinned by the live
                # pre-accumulation tiles (allocated h10) and the sp/op rings
                # are fully used by the head itself; ~80% PE duty keeps HAM
                # awake without dummies.
            }
            for h in range(H):
                head(h, FILL.get(h))
                if h == 0:  # OS head0-ic0 now exists -> release late bulk
                    gated_w(WQ, wqr, 3, OS[0:1, 0:1])
                    gated_w(WK, wkr, 3, OS[0:1, 0:1])
                    gated_wv(3, OS[0:1, 0:1])
                    gated_dma(WO[:], wo, WO[0:1, 0:1], OS[0:1, 0:1])
                elif h == 1:
                    gated_w(WQ, wqr, 4, OS[64:65, 0:1])
                    gated_w(WK, wkr, 4, OS[64:65, 0:1])
                elif h == 2:
                    gated_w(WQ, wqr, 5, OS[0:1, T : T + 1])
                    gated_w(WK, wkr, 5, OS[0:1, T : T + 1])

            # bridge the final-normalize window (scores ring: the proj ring
            # holds the live pre-accumulated it=0 tiles)
            for _ in range(6):
                dummy_sc()

            # ---- output projection ----
            fin0 = fpool.tile([128, E], f32, tag="fin")
            for f_ps, n0, nw in pre:
                oproj_chunk(0, n0, nw, f_ps, [NT - 1], False, True)
                nc.vector.tensor_add(
                    fin0[:, n0 : n0 + nw], f_ps[:, :nw], FINB[:, n0 : n0 + nw]
                )
                nc.sync.dma_start(out[0:128, n0 : n0 + nw], fin0[:, n0 : n0 + nw])
            for it in range(1, NJ):
                fin = fpool.tile([128, E], f32, tag="fin")
                for n0, nw in ((0, 512), (512, 256)):
                    f_ps = pp.tile([128, 512], f32, tag="proj")
                    oproj_chunk(it, n0, nw, f_ps, range(NT), True, True)
                    nc.vector.tensor_add(
                        fin[:, n0 : n0 + nw], f_ps[:, :nw], FINB[:, n0 : n0 + nw]
                    )
                    # alternate output chunks across both HW-DGE queues so
                    # the final drain isn't serialized on one queue
                    eng = nc.sync if it % 2 == 0 else nc.scalar
                    eng.dma_start(
                        out[it * 128 : (it + 1) * 128, n0 : n0 + nw],
                        fin[:, n0 : n0 + nw],
                    )

    nc.compile()
    return nc


def _get_nc():
    if "nc" not in _CACHE:
        _CACHE["nc"] = _build()
    return _CACHE["nc"]


def _make_in_maps(inputs):
    bf = ml_dtypes.bfloat16
    x = np.asarray(inputs["x"], np.float32)

    def nt_major(w):
        # row nt*128+p, col kt*128+c = W[kt*128+p, nt*128+c]
        w4 = np.asarray(w, np.float32).reshape(NT, 128, NT, 128)  # [kt,p,nt,c]
        return np.ascontiguousarray(w4.transpose(2, 1, 0, 3).reshape(E, E)).astype(bf)

    def kt_major(w):
        # [p, kt*768+e] = W[kt*128+p, e]
        w3 = np.asarray(w, np.float32).reshape(NT, 128, E)  # [kt,p,e]
        return np.ascontiguousarray(w3.transpose(1, 0, 2).reshape(128, NT * E)).astype(bf)

    def chunk_major(w):
        # concat per-chunk blocks [p, kt*ew + c] = W[kt*128+p, e0+c]
        w3 = np.asarray(w, np.float32).reshape(NT, 128, E)  # [kt,p,e]
        blocks = []
        for e0, ew in ((0, 128), (128, 128), (256, 256), (512, 256)):
            blocks.append(
                w3[:, :, e0 : e0 + ew].transpose(1, 0, 2).reshape(128, NT * ew)
            )
        return np.ascontiguousarray(np.concatenate(blocks, axis=1)).astype(bf)

    shared = {
        "wq": nt_major(inputs["Wq"]),
        "wk": nt_major(inputs["Wk"]),
        "wv": chunk_major(inputs["Wv"]),
        "wo": kt_major(inputs["Wo"]),
        "bqt": np.ascontiguousarray(
            np.asarray(inputs["bq"], np.float32).reshape(NT, 128).T
        ),
        "bkt": np.ascontiguousarray(
            np.asarray(inputs["bk"], np.float32).reshape(NT, 128).T
        ),
        "bvr": np.asarray(inputs["bv"], np.float32).reshape(1, E).astype(bf),
        "bor": np.asarray(inputs["bo"], np.float32).reshape(1, E).astype(bf),
        "tri": np.triu(np.ones((128, 128), np.float32)).astype(bf),
    }
    return [dict(shared, xt=x[b].T.astype(bf)) for b in range(B)]


def _run(inputs, trace=False):
    from concourse import bass_utils

    nc = _get_nc()
    res = bass_utils.run_bass_kernel_spmd(
        nc, _make_in_maps(inputs), core_ids=list(range(B)), trace=trace
    )
    out = np.stack([np.asarray(res.results[c]["out"]) for c in range(B)])
    return out, res


def kernel(**inputs) -> np.ndarray:
    out, _ = _run(inputs, trace=False)
    return out
